# revision 49
# baseline (speedup 1.0000x reference)
"""Multi-head self-attention (B=2, S=2048, D=1024, H=16) on 8 trn2 cores.

Sharding: core c = b*4 + g  (b = batch, g = head-group of 4 heads).
Each core computes, for its batch b and heads 4g..4g+3:
  Qt/Kt = W^T x_b^T + bias   -> [128, 2048] feature-major per head pair
  V||ones                    -> vstk [128 keys, h, cp, 65]
  scoresT[k,q] per head      -> [128 keys, 1024 q] psum ([64,128] Kt weights)
  expT = exp(0.125*scoresT)  (ACT, straight from PSUM)
  ctxT/rowsum via PV matmul with [128, 65] V||ones weights (contract 128 keys)
  ctxT normalized by 1/rowsum (DMA partition-broadcast + DVE mult)
  y = ctx @ Wo               -> [2048, 1024]
One globally software-pipelined attention loop (128 iterations); pair-1
Q/K projections and the q-half0 output projection interleave into the
exp stream as weight-load-paired feeder jobs; the q-half1 output
projection rides the freed sc-tag PSUM rotation in the tail. The last
section's normalize chain is PE/DVE-only (rowsum-slab transposes ->
128-lane reciprocal -> transpose back -> one-hot-weight broadcast
matmuls) instead of the exposed 3-hop elementwise-DMA chain used for
hidden mid-stream sections. Host: Y[b] = sum_g y_partial + (bo + bv@Wo).
"""

import sys

sys.path.insert(0, "/opt/trn_rl_repo")

import numpy as np

import concourse.bass as bass
import concourse.mybir as mybir
import concourse.tile as tile

F32 = mybir.dt.float32
F16 = mybir.dt.float16
BF16 = mybir.dt.bfloat16
MMDT = F16                     # dtype for all matmul operands
AF = mybir.ActivationFunctionType

D = 1024          # d_model
S = 2048          # sequence length
HPC = 4           # heads per core
DK = 64           # head dim
E = HPC * DK      # 256 features per core
N_CORES = 8

KT = D // 128     # 8 k-tiles over d_model
CP = S // 128     # 16 key chunks of 128
ET = 2            # head pairs

# attention sections: (head, q-half), qh-outer
SECTIONS = [(h, qh) for qh in range(2) for h in range(HPC)]
NG = len(SECTIONS) * CP        # 128 global iterations


_ENGINE_OPS = {
    "InstMatmult", "InstActivation", "InstTensorCopy", "InstTensorTensor",
    "InstReciprocal", "InstTensorReduce", "InstMemset", "InstIota",
    "InstTensorScalarPtr", "InstTranspose", "InstLdweights",
    "InstDMACopy", "InstDmaTransposeAnt", "InstDrain", "InstNoOp",
}


def _dedup_ldweights(nc):
    """Consecutive matmuls often reuse identical weights (score j-halves, PV
    j-halves, proj q-chunks). The rust layer emits one standalone
    InstLdweights per matmul; reloading identical weights costs ~100ns of PE
    array drain each. Drop an InstLdweights when the previously loaded
    weights AP is byte-identical (weight tiles here are write-once), folding
    its waits into the next matmul (legalized afterwards)."""

    def key(ap):
        return (ap.memref, ap.offset, str(ap.ap), str(ap.dtype))

    n_drop = 0
    for f in nc.m.functions:
        for bb in f.blocks:
            out = []
            pending_waits = []
            last_w = None
            for i in bb.instructions:
                if type(i).__name__ == "InstLdweights":
                    k = key(i.ins[0])
                    if k == last_w:
                        si = getattr(i, "sync_info", None)
                        if si is not None and si.on_wait:
                            pending_waits.extend(si.on_wait)
                        n_drop += 1
                        continue
                    last_w = k
                elif (
                    type(i).__name__ == "InstMatmult"
                    and len(i.ins) > 1
                    and "float32" in str(getattr(i.ins[1], "dtype", ""))
                ):
                    # self-loading fp32 matmul clobbers the PE stationary
                    # registers; forget the cached ldweights AP
                    last_w = None
                    if pending_waits:
                        si = getattr(i, "sync_info", None)
                        waits = list(si.on_wait) if si else []
                        upd = list(si.on_update) if si else []
                        i.sync_info = mybir.SyncInfo(
                            on_wait=pending_waits + waits, on_update=upd)
                        pending_waits = []
                elif pending_waits and type(i).__name__ == "InstMatmult":
                    si = getattr(i, "sync_info", None)
                    waits = list(si.on_wait) if si else []
                    upd = list(si.on_update) if si else []
                    i.sync_info = mybir.SyncInfo(
                        on_wait=pending_waits + waits, on_update=upd)
                    pending_waits = []
                out.append(i)
            assert not pending_waits
            bb.instructions = out
    return n_drop


def _legalize_matmul_waits(nc):
    """walrus allows at most 1 sync wait on engine compute instructions; Tile
    sometimes emits more. Move the excess onto EventSemaphore instructions
    (cap 2 each) placed immediately before in same-engine program order."""
    for f in nc.m.functions:
        for bb in f.blocks:
            out = []
            changed = False
            for i in bb.instructions:
                si = getattr(i, "sync_info", None)
                if (
                    type(i).__name__ in _ENGINE_OPS
                    and si is not None
                    and si.on_wait
                    and len(si.on_wait) > 1
                ):
                    waits = list(si.on_wait)
                    excess, keep = waits[:-1], waits[-1:]
                    for c in range(0, len(excess), 2):
                        ev = mybir.InstEventSemaphore(
                            name=f"{i.name}-mmw{c}", ins=[], outs=[]
                        )
                        ev.engine = i.engine
                        ev.sync_info = mybir.SyncInfo(
                            on_wait=excess[c:c + 2], on_update=[]
                        )
                        out.append(ev)
                    i.sync_info = mybir.SyncInfo(
                        on_wait=keep, on_update=list(si.on_update)
                    )
                    changed = True
                out.append(i)
            if changed:
                bb.instructions = out


def build_nc():
    nc = bass.Bass()

    xt = nc.dram_tensor("xt", [D, S], MMDT, kind="ExternalInput")
    wq = nc.dram_tensor("wq", [D, E], MMDT, kind="ExternalInput")
    wk = nc.dram_tensor("wk", [D, E], MMDT, kind="ExternalInput")
    wv = nc.dram_tensor("wv", [D, E], MMDT, kind="ExternalInput")
    wo = nc.dram_tensor("wo", [E, D], MMDT, kind="ExternalInput")
    bq = nc.dram_tensor("bq", [E], F32, kind="ExternalInput")
    bk = nc.dram_tensor("bk", [E], F32, kind="ExternalInput")
    # bf16 output halves y DMA traffic (the post-attention drain);
    # host accumulates in f32. ~2e-3 rel err, tolerance is 2e-2.
    y = nc.dram_tensor("y", [S, D], BF16, kind="ExternalOutput")

    with tile.TileContext(nc) as tc:
        with tc.tile_pool(name="persist", bufs=1) as pp:
            # ---- persistent tiles ----
            qt_sb = [pp.tile([128, S], MMDT, tag=f"qt{t}", name=f"qt{t}")
                     for t in range(ET)]
            ktp = [pp.tile([128, S], MMDT, tag=f"ktp{t}", name=f"ktp{t}")
                   for t in range(ET)]
            # vstk[:, h, cp, 0:64] = V rows for head h, keys cp*128..+128
            # (partition = key within chunk); [:, h, cp, 64] = 1.0 (rowsum).
            vstk = pp.tile([128, HPC, CP, DK + 1], MMDT, tag="vstk")
            ctx_sb = [pp.tile([128, S], MMDT, tag=f"ctx{t}", name=f"ctx{t}")
                      for t in range(ET)]
            wo_sb = [pp.tile([128, D], MMDT, tag=f"wo{t}", name=f"wo{t}")
                     for t in range(ET)]
            bq_sb = pp.tile([128, ET], F32, tag="bq")
            bk_sb = pp.tile([128, ET], F32, tag="bk")
            xt_sb = [pp.tile([128, S], MMDT, tag=f"xt{k}", name=f"xt{k}")
                     for k in range(KT)]
            wq_sb = pp.tile([128, KT, E], MMDT, tag="wq")
            wk_sb = pp.tile([128, KT, E], MMDT, tag="wk")
            wv_sb = pp.tile([128, KT, E], MMDT, tag="wv")

            # k-layer inputs arrive together so QK0 proj chases the DMAs;
            # xt split over three queues per chunk, wq/wk as single full-E
            # descriptors (512B rows beat 2x256B for packet throughput).
            # (bq/bk issue after the first xt chunk - needed only at ~20us)
            for k in range(KT):
                if k == 0:
                    # pair-0 slice first so the k=0 ldweights fires early
                    nc.scalar.dma_start(wq_sb[:, k, 0:128],
                                        wq[0:128, 0:128])
                    nc.scalar.dma_start(wq_sb[:, k, 128:256],
                                        wq[0:128, 128:256])
                else:
                    nc.scalar.dma_start(wq_sb[:, k, :],
                                        wq[k * 128:(k + 1) * 128, :])
                nc.scalar.dma_start(wk_sb[:, k, :],
                                    wk[k * 128:(k + 1) * 128, :])
                nc.scalar.dma_start(wv_sb[:, k, :],
                                    wv[k * 128:(k + 1) * 128, :])
                if k == 0:
                    nc.sync.dma_start(xt_sb[k][:, 0:512],
                                      xt[k * 128:(k + 1) * 128, 0:512])
                    nc.sync.dma_start(xt_sb[k][:, 512:1024],
                                      xt[k * 128:(k + 1) * 128, 512:1024])
                else:
                    nc.sync.dma_start(xt_sb[k][:, 0:1024],
                                      xt[k * 128:(k + 1) * 128, 0:1024])
                nc.gpsimd.dma_start(xt_sb[k][:, 1024:2048],
                                    xt[k * 128:(k + 1) * 128, 1024:2048])
            # bq/bk are tiny elementwise scatters; keep them off the xt
            # path (needed only at the first QK eviction, ~30us)
            nc.gpsimd.dma_start(bq_sb, bq.rearrange("(t p) -> p t", p=128))
            nc.gpsimd.dma_start(bk_sb, bk.rearrange("(t p) -> p t", p=128))
            # warm the ACT exp table while DMAs stream: the compiler
            # statically inserts the 1.3us EXP table load before the first
            # exp-consuming ACT instruction; this dummy (queued before the
            # wv/wo DMA issues) pulls that load off the phase-A critical
            # path.
            ones_sb2 = pp.tile([128, CP], F32, tag="ones2")
            nc.vector.memset(ones_sb2, 1.0)
            tbl_warm = pp.tile([128, 16], MMDT, tag="tblw")
            nc.scalar.activation(tbl_warm, ones_sb2[:, 0:16], AF.Exp,
                                 scale=0.125)
            for t in range(ET):
                nc.scalar.dma_start(wo_sb[t], wo[t * 128:(t + 1) * 128, :])

            ones_sb = pp.tile([128, CP], F32, tag="ones")
            nc.vector.memset(ones_sb, 1.0)
            for h in range(HPC):
                nc.vector.tensor_copy(vstk[:, h, :, DK:DK + 1],
                                      ones_sb[:, :, None])
            # one-time tiles for the final-section PE-based normalize chain
            # (see section_end): identities for the two transposes (id32
            # also copied to base partition 64 - matmul requires lhsT/rhs
            # at the same base), per-chunk one-hot weights for the
            # q-broadcast matmul, and staging tiles.
            id32 = pp.tile([32, 32], MMDT, tag="id32")
            nc.vector.memset(id32, 1.0)
            nc.gpsimd.affine_select(id32, id32, pattern=[[-1, 32]],
                                    compare_op=mybir.AluOpType.is_equal,
                                    fill=0.0, base=0, channel_multiplier=1)
            id32_hi = pp.tile([96, 32], MMDT, tag="id32hi")
            nc.sync.dma_start(id32_hi[64:96, :], id32)
            id128 = pp.tile([128, 128], MMDT, tag="id128")
            nc.vector.memset(id128, 1.0)
            nc.gpsimd.affine_select(id128, id128, pattern=[[-1, 128]],
                                    compare_op=mybir.AluOpType.is_equal,
                                    fill=0.0, base=0, channel_multiplier=1)
            onesW = pp.tile([DK, 8 * DK], MMDT, tag="onesW")
            nc.vector.memset(onesW, 1.0)
            nc.gpsimd.affine_select(
                onesW.rearrange("p (c f) -> p c f", f=DK),
                onesW.rearrange("p (c f) -> p c f", f=DK),
                pattern=[[-1, 8], [0, DK]],
                compare_op=mybir.AluOpType.is_equal,
                fill=0.0, base=0, channel_multiplier=1)
            rinvT_sb = pp.tile([128, 128], MMDT, tag="rinvT")
            nc.vector.memset(rinvT_sb, 0.0)
            # final-section staging: ctx rows 0:64 + rowsum row 64; rows
            # 65:96 zeroed so the 32-row transpose slab is NaN-free.
            stgF = pp.tile([96, 1024], MMDT, tag="stg16")
            # rows 64:96 zeroed at setup; row 64 is overwritten with the
            # rowsum by the final section's staging copies.
            nc.vector.memset(stgF[DK:96, :], 0.0)

            # ---- stage A: pair-0 Q/K projections + V (all heads) ----
            with tc.tile_pool(name="psA", bufs=4, space="PSUM") as psA:
                # 4 resident [128,1024] psums (8 banks), k-outer so matmuls
                # chase the input DMAs.
                qps = [psA.tile([128, 1024], F32, tag="proj", name=f"qp{i}")
                       for i in range(2)]
                kps = [psA.tile([128, 1024], F32, tag="proj", name=f"kp{i}")
                       for i in range(2)]
                for k in range(KT):
                    for ps, w_sb in ((qps, wq_sb), (kps, wk_sb)):
                        for half in range(2):
                            for j in range(2):
                                q0 = half * 1024 + j * 512
                                nc.tensor.matmul(
                                    ps[half][:, j * 512:(j + 1) * 512],
                                    w_sb[:, k, 0:128],
                                    xt_sb[k][:, q0:q0 + 512],
                                    start=(k == 0), stop=(k == KT - 1),
                                )
                # evictions with bias on DVE (idle during stage A; ACT
                # carries the prefetched exp-table load)
                for half in range(2):
                    nc.vector.tensor_scalar_add(
                        qt_sb[0][:, half * 1024:(half + 1) * 1024],
                        qps[half], bq_sb[:, 0:1])
                for half in range(2):
                    nc.vector.tensor_scalar_add(
                        ktp[0][:, half * 1024:(half + 1) * 1024],
                        kps[half], bk_sb[:, 0:1])

                # V for all 4 heads, seq-major: psum [128 seq, 256 feat].
                # (A feature-major V + DMA-xbar transpose into vstk was
                # tried - fewer ldweights - but the xbar's output run
                # ordering for free dims > 256 doesn't match its documented
                # tile layout, and stage A is HBM-bound anyway.)
                for s in range(CP):
                    vp = psA.tile([128, 1024], F32, tag="proj",
                                  name=f"vp{s}")
                    for k in range(KT):
                        nc.tensor.matmul(
                            vp[:, 0:E],
                            xt_sb[k][:, s * 128:(s + 1) * 128],
                            wv_sb[:, k, :],
                            start=(k == 0), stop=(k == KT - 1),
                        )
                    nc.vector.tensor_copy(
                        vstk[:, :, s, 0:DK],
                        vp[:, 0:E].rearrange("p (h d) -> p h d", d=DK))

            # ---- stage B: one pipelined attention loop ----
            pb = pp  # stage-B sbuf tiles live in the persist pool
            with (
                tc.tile_pool(name="dramB", bufs=3, space="DRAM") as dramB,
            ):
              with tc.tile_pool(name="psB", bufs=2,
                                space="PSUM") as psB:
                psS = psC = psX = psB
                # --- feeder jobs: closures emitting PE matmuls ---
                def proj1_jobs(w_sb, b_col, dst, pairs):
                    """Pair-1 proj matmuls, k-inner per [128,1024] chunk
                    pair; the two 512-col matmuls per k share one
                    ldweights."""
                    jobs = []
                    for cp0 in pairs:
                        ps = {}

                        def mk(k, cp0=cp0, ps=ps):
                            def go():
                                if k == 0:
                                    ps[0] = psX.tile(
                                        [128, 1024], F32, tag="aux",
                                        name=f"pj{id(w_sb)}_{cp0}", bufs=1)
                                for j in range(2):
                                    nc.tensor.matmul(
                                        ps[0][:, j * 512:(j + 1) * 512],
                                        w_sb[:, k, 128:256],
                                        xt_sb[k][:, cp0 * 1024 + j * 512:
                                                  cp0 * 1024 + (j + 1) * 512],
                                        start=(k == 0), stop=(k == KT - 1),
                                    )
                                if k == KT - 1:
                                    nc.vector.tensor_scalar_add(
                                        dst[:, cp0 * 1024:(cp0 + 1) * 1024],
                                        ps[0], b_col)
                            return go
                        for k in range(KT):
                            jobs.append(mk(k))
                    return jobs

                def outproj_jobs(tiles):
                    """Output projection per 128-q tile: 2 jobs of 2 matmuls
                    (both n-halves share the ctx-chunk ldweights) + evict +
                    y DMA."""
                    jobs = []
                    for qt_i in tiles:
                        st = {}

                        def mk(t, qt_i=qt_i, st=st):
                            def go():
                                if t == 0:
                                    st['ps'] = psX.tile(
                                        [128, 1024], F32, tag="aux",
                                        name=f"yp{qt_i}", bufs=1)
                                    st['ys'] = pb.tile(
                                        [128, 1024], BF16, tag="ys",
                                        name=f"ys{qt_i}", bufs=6)
                                for n in range(2):
                                    nc.tensor.matmul(
                                        st['ps'][:, n * 512:(n + 1) * 512],
                                        ctx_sb[t][:,
                                                  qt_i * 128:(qt_i + 1) * 128],
                                        wo_sb[t][:, n * 512:(n + 1) * 512],
                                        start=(t == 0), stop=(t == ET - 1),
                                    )
                                if t == ET - 1:
                                    nc.vector.tensor_copy(st['ys'], st['ps'])
                                    nc.sync.dma_start(
                                        y[qt_i * 128:(qt_i + 1) * 128, :],
                                        st['ys'])
                            return go
                        for t in range(ET):
                            jobs.append(mk(t))
                    return jobs

                # schedule feeder jobs onto global iterations
                schedule = [[] for _ in range(NG)]

                def assign(g_lo, g_hi, jobs):
                    n_slots = g_hi - g_lo
                    acc = 0.0
                    i = 0
                    per = len(jobs) / n_slots
                    for g in range(g_lo, g_hi):
                        acc += per
                        take = int(round(acc)) - i
                        schedule[g] = jobs[i:i + take]
                        i += take
                    assert i == len(jobs)

                # Q1 half0 + K1 during sections (0,0),(1,0); order matters:
                # section (2,0)'s first scores (emitted at g=31) need the
                # Q1-h0 and K1-ch0 evictions done well before.
                assign(0, 26,
                       proj1_jobs(wq_sb, bq_sb[:, 1:2], qt_sb[1], [0])
                       + proj1_jobs(wk_sb, bk_sb[:, 1:2], ktp[1], [0]))
                # K1 half1 needed from g=40 ((2,0) cp=8); Q1 half1 from
                # g=95 - flatter spread than packing everything in [0,32)
                assign(26, 37,
                       proj1_jobs(wk_sb, bk_sb[:, 1:2], ktp[1], [1]))
                assign(37, 72,
                       proj1_jobs(wq_sb, bq_sb[:, 1:2], qt_sb[1], [1]))
                # outproj q-half0, all 8 tiles, during (0,1)..(2,1); offset
                # past (3,0)'s normalize chain (its ctx lands a few us into
                # (0,1)).
                assign(72, 126, outproj_jobs(range(8)))

                def emit_scores(g):
                    sec, cp = divmod(g, CP)
                    h, qh = SECTIONS[sec]
                    t, hp = h // 2, h % 2
                    sc_ps = psS.tile([128, 1024], F32, tag="sc",
                                     name=f"sc{g}")
                    for j in range(2):
                        nc.tensor.matmul(
                            sc_ps[:, j * 512:(j + 1) * 512],
                            ktp[t][hp * 64:hp * 64 + 64,
                                   cp * 128:(cp + 1) * 128],
                            qt_sb[t][hp * 64:hp * 64 + 64,
                                     qh * 1024 + j * 512:
                                     qh * 1024 + (j + 1) * 512],
                            start=True, stop=True,
                        )
                    return sc_ps

                def section_end(h, qh, ctx_ps, final=False):
                    t, hp = h // 2, h % 2
                    if final:
                        # Last section: the DMA-hop chain below would sit
                        # exposed on the critical path (~13us of elementwise
                        # DMA latency in a congested tail). PE/DVE-only
                        # instead: transpose the 32-row rowsum slab (row 64
                        # = rowsum, 65:96 zeros) onto q-partitions,
                        # reciprocal on 128 lanes, transpose back to a row
                        # layout, then contraction-64 matmuls with one-hot
                        # weights broadcast rinv[q] to the 64 ctx
                        # partitions.
                        for j in range(2):
                            nc.vector.tensor_copy(
                                stgF[0:DK + 1, j * 512:(j + 1) * 512],
                                ctx_ps[j])
                        rsT = psX.tile([128, 1024], MMDT, tag="aux",
                                       name="rsT", bufs=1)
                        for c in range(8):
                            nc.tensor.transpose(
                                rsT[:, c * 32:(c + 1) * 32],
                                stgF[DK:DK + 32, c * 128:(c + 1) * 128],
                                id32_hi[DK:DK + 32, :])
                        rinvq = pb.tile([128, 8], MMDT, tag="rinvq")
                        with nc.allow_low_precision("fp16 rinv, tol 2e-2"):
                            nc.vector.reciprocal(
                                rinvq[:, :, None],
                                rsT[:, 0:256].rearrange(
                                    "p (c w) -> p c w", w=32)[:, :, 0:1])
                        rinvT_ps = psX.tile([128, 1024], MMDT, tag="aux",
                                            name="rinvT_ps", bufs=1)
                        nc.tensor.transpose(rinvT_ps[0:8, 0:128], rinvq,
                                            id128)
                        nc.vector.tensor_copy(rinvT_sb[0:8, :],
                                              rinvT_ps[0:8, 0:128])
                        rb_ps = psX.tile([128, 1024], F32, tag="aux",
                                         name="rbps", bufs=1)
                        for c in range(8):
                            nc.tensor.matmul(
                                rb_ps[0:DK, c * 128:(c + 1) * 128],
                                onesW[:, c * DK:(c + 1) * DK],
                                rinvT_sb[0:DK, 0:128],
                                start=True, stop=True)
                        for j in range(2):
                            nc.vector.tensor_mul(
                                ctx_sb[t][hp * 64:hp * 64 + 64,
                                          qh * 1024 + j * 512:
                                          qh * 1024 + (j + 1) * 512],
                                stgF[0:DK, j * 512:(j + 1) * 512],
                                rb_ps[0:DK, j * 512:(j + 1) * 512],
                            )
                        return
                    stg = pb.tile([DK + 1, 1024], F32, tag="stg",
                                  name=f"stg{h}_{qh}", bufs=3)
                    for j in range(2):
                        nc.vector.tensor_copy(
                            stg[:, j * 512:(j + 1) * 512], ctx_ps[j])
                    # reciprocal of rowsum via DRAM scatter to 64 partitions
                    # ([1,1024] single-lane DVE reciprocal is ~6.5us; this
                    # chain is ~3us and hidden under the next section).
                    rc_dr = dramB.tile([1, 1024], F32, tag="rc_dr",
                                       name=f"rcdr{h}_{qh}")
                    # single SBUF->SBUF partition-scatter DMA (dst/src APs
                    # iterate element-wise) replaces the two-hop DRAM bounce
                    rs64 = pb.tile([64, 16], F32, tag="rs64",
                                   name=f"rs64{h}_{qh}", bufs=3)
                    nc.sync.dma_start(rs64, stg[DK:DK + 1, :])
                    rc64 = pb.tile([64, 16], F32, tag="rc64",
                                   name=f"rc64{h}_{qh}", bufs=3)
                    nc.vector.reciprocal(rc64, rs64)
                    nc.sync.dma_start(
                        rc_dr.rearrange("o (p f) -> (o p) f", f=16), rc64)
                    rb = pb.tile([64, 1024], F32, tag="rb",
                                 name=f"rb{h}_{qh}", bufs=3)
                    # halves pipelined: first mul starts after half the
                    # broadcast transfer (matters for the final section,
                    # whose chain gates the tail)
                    for j in range(2):
                        nc.sync.dma_start(
                            rb[:, j * 512:(j + 1) * 512],
                            rc_dr[:, j * 512:(j + 1) * 512]
                            .to_broadcast([64, 512]))
                        nc.vector.tensor_mul(
                            ctx_sb[t][hp * 64:hp * 64 + 64,
                                      qh * 1024 + j * 512:
                                      qh * 1024 + (j + 1) * 512],
                            stg[0:DK, j * 512:(j + 1) * 512],
                            rb[:, j * 512:(j + 1) * 512],
                        )

                ctx_ps = None
                sc_cur = emit_scores(0)
                for g in range(NG):
                    sec, cp = divmod(g, CP)
                    h, qh = SECTIONS[sec]
                    if cp == 0:
                        ctx_ps = [psC.tile([DK + 1, 512], F32, tag="ctx",
                                           name=f"cx{sec}_{j}")
                                  for j in range(2)]
                    sc_next = emit_scores(g + 1) if g + 1 < NG else None
                    ex = pb.tile([128, 1024], MMDT, tag="ex",
                                 name=f"ex{g}", bufs=6)
                    nc.scalar.activation(ex, sc_cur, AF.Exp, scale=0.125)
                    for job in schedule[g]:
                        job()
                    for j in range(2):
                        nc.tensor.matmul(
                            ctx_ps[j],
                            vstk[:, h, cp, :],
                            ex[:, j * 512:(j + 1) * 512],
                            start=(cp == 0), stop=(cp == CP - 1),
                        )
                    if cp == CP - 1 and g < NG - 1:
                        section_end(h, qh, ctx_ps)
                    sc_cur = sc_next

                # ---- tail: output projection for q-half 1, inside the
                # psB scope so y psums ride the freed sc-tag rotation.
                # Tiles 8-9's t=0 matmuls are emitted before the final
                # normalize chain so the PE stays busy (and in its high
                # p-state) while the chain's reciprocal resolves; tiles
                # 10-15 then run full psum chains.
                def t_mm(yp, t, qt_i, start, stop):
                    for n in range(2):
                        nc.tensor.matmul(
                            yp[:, n * 512:(n + 1) * 512],
                            ctx_sb[t][:, qt_i * 128:(qt_i + 1) * 128],
                            wo_sb[t][:, n * 512:(n + 1) * 512],
                            start=start, stop=stop,
                        )

                yps = {}
                for qt_i in range(8, 10):
                    yp = psS.tile([128, 1024], F32, tag="sc",
                                  name=f"yt{qt_i}")
                    yps[qt_i] = yp
                    t_mm(yp, 0, qt_i, start=True, stop=False)

                section_end(3, 1, ctx_ps, final=True)

                def evict_dma(qt_i, yp, i):
                    ys = pb.tile([128, 1024], BF16, tag="ys",
                                 name=f"ys{qt_i}", bufs=6)
                    if i % 2 == 0:
                        nc.scalar.copy(ys, yp)
                    else:
                        nc.vector.tensor_copy(ys, yp)
                    rows = slice(qt_i * 128, (qt_i + 1) * 128)
                    qs = (nc.sync, nc.gpsimd, nc.scalar)
                    qs[qt_i % 3].dma_start(y[rows, 0:512], ys[:, 0:512])
                    qs[(qt_i + 1) % 3].dma_start(y[rows, 512:1024],
                                                 ys[:, 512:1024])

                for i, qt_i in enumerate(range(8, 10)):
                    yp = yps[qt_i]
                    t_mm(yp, 1, qt_i, start=False, stop=True)
                    evict_dma(qt_i, yp, i)
                for i, qt_i in enumerate(range(10, 15)):
                    yp = psS.tile([128, 1024], F32, tag="sc",
                                  name=f"yt{qt_i}")
                    t_mm(yp, 0, qt_i, start=True, stop=False)
                    t_mm(yp, 1, qt_i, start=False, stop=True)
                    evict_dma(qt_i, yp, i)
                # last tile: evict+DMA per 512-col half on separate engines
                # so the final transfer starts as early as possible
                yp = psS.tile([128, 1024], F32, tag="sc", name="yt15")
                t_mm(yp, 0, 15, start=True, stop=False)
                t_mm(yp, 1, 15, start=False, stop=True)
                ys15 = pb.tile([128, 1024], BF16, tag="ys", name="ys15",
                               bufs=6)
                nc.scalar.copy(ys15[:, 0:512], yp[:, 0:512])
                nc.sync.dma_start(y[15 * 128:2048, 0:512], ys15[:, 0:512])
                nc.vector.tensor_copy(ys15[:, 512:1024], yp[:, 512:1024])
                nc.gpsimd.dma_start(y[15 * 128:2048, 512:1024],
                                    ys15[:, 512:1024])
    _dedup_ldweights(nc)
    _legalize_matmul_waits(nc)
    return nc


_NC_CACHE = None


def _get_nc():
    global _NC_CACHE
    if _NC_CACHE is None:
        _NC_CACHE = build_nc()
    return _NC_CACHE


def make_in_maps(inputs):
    mmnp = mybir.dt.np(MMDT)
    x = np.asarray(inputs["x"], dtype=np.float32)
    Wq = np.asarray(inputs["Wq"], dtype=np.float32)
    Wk = np.asarray(inputs["Wk"], dtype=np.float32)
    Wv = np.asarray(inputs["Wv"], dtype=np.float32)
    Wo = np.asarray(inputs["Wo"], dtype=np.float32)
    bq = np.asarray(inputs["bq"], dtype=np.float32)
    bk = np.asarray(inputs["bk"], dtype=np.float32)

    in_maps = []
    for c in range(N_CORES):
        b, g = c // 4, c % 4
        sl = slice(g * E, (g + 1) * E)
        in_maps.append({
            "xt": np.ascontiguousarray(x[b].T).astype(mmnp),
            "wq": np.ascontiguousarray(Wq[:, sl]).astype(mmnp),
            "wk": np.ascontiguousarray(Wk[:, sl]).astype(mmnp),
            "wv": np.ascontiguousarray(Wv[:, sl]).astype(mmnp),
            "wo": np.ascontiguousarray(Wo[sl, :]).astype(mmnp),
            "bq": np.ascontiguousarray(bq[sl]),
            "bk": np.ascontiguousarray(bk[sl]),
        })
    return in_maps


def kernel(x, Wq, bq, Wk, bk, Wv, bv, Wo, bo):
    from concourse.bass_utils import run_bass_kernel_spmd

    x = np.asarray(x, dtype=np.float32)
    Wv = np.asarray(Wv, dtype=np.float32)
    Wo = np.asarray(Wo, dtype=np.float32)
    bv = np.asarray(bv, dtype=np.float32)
    bo = np.asarray(bo, dtype=np.float32)

    B = x.shape[0]
    nc = _get_nc()
    in_maps = make_in_maps({
        "x": x, "Wq": Wq, "Wk": Wk, "Wv": Wv, "Wo": Wo, "bq": bq, "bk": bk,
    })

    res = run_bass_kernel_spmd(nc, in_maps, core_ids=list(range(N_CORES)))

    bias_total = bo + bv @ Wo  # [D]
    out = np.zeros((B, S, D), dtype=np.float32)
    for c in range(N_CORES):
        out[c // 4] += np.asarray(res.results[c]["y"], dtype=np.float32)
    out += bias_total[None, None, :]
    return out



# revision 50
# speedup vs baseline: 1.0089x; 1.0089x over previous
"""Multi-head self-attention (B=2, S=2048, D=1024, H=16) on 8 trn2 cores.

Sharding: core c = b*4 + g  (b = batch, g = head-group of 4 heads).
Each core computes, for its batch b and heads 4g..4g+3:
  Qt/Kt = W^T x_b^T + bias   -> [128, 2048] feature-major per head pair
  V||ones                    -> vstk [128 keys, h, cp, 65]
  scoresT[k,q] per head      -> [128 keys, 1024 q] psum ([64,128] Kt weights)
  expT = exp(0.125*scoresT)  (ACT, straight from PSUM)
  ctxT/rowsum via PV matmul with [128, 65] V||ones weights (contract 128 keys)
  ctxT normalized by 1/rowsum (DMA partition-broadcast + DVE mult)
  y = ctx @ Wo               -> [2048, 1024]
One globally software-pipelined attention loop (128 iterations); pair-1
Q/K projections and the q-half0 output projection interleave into the
exp stream as weight-load-paired feeder jobs; the q-half1 output
projection rides the freed sc-tag PSUM rotation in the tail. The last
section's normalize chain is PE/DVE-only (rowsum-slab transposes ->
128-lane reciprocal -> transpose back -> one-hot-weight broadcast
matmuls) instead of the exposed 3-hop elementwise-DMA chain used for
hidden mid-stream sections. Host: Y[b] = sum_g y_partial + (bo + bv@Wo).
"""

import sys

sys.path.insert(0, "/opt/trn_rl_repo")

import numpy as np

import concourse.bass as bass
import concourse.mybir as mybir
import concourse.tile as tile

F32 = mybir.dt.float32
F16 = mybir.dt.float16
BF16 = mybir.dt.bfloat16
MMDT = F16                     # dtype for all matmul operands
AF = mybir.ActivationFunctionType

D = 1024          # d_model
S = 2048          # sequence length
HPC = 4           # heads per core
DK = 64           # head dim
E = HPC * DK      # 256 features per core
N_CORES = 8

KT = D // 128     # 8 k-tiles over d_model
CP = S // 128     # 16 key chunks of 128
ET = 2            # head pairs

# attention sections: (head, q-half), qh-outer
SECTIONS = [(h, qh) for qh in range(2) for h in range(HPC)]
NG = len(SECTIONS) * CP        # 128 global iterations


_ENGINE_OPS = {
    "InstMatmult", "InstActivation", "InstTensorCopy", "InstTensorTensor",
    "InstReciprocal", "InstTensorReduce", "InstMemset", "InstIota",
    "InstTensorScalarPtr", "InstTranspose", "InstLdweights",
    "InstDMACopy", "InstDmaTransposeAnt", "InstDrain", "InstNoOp",
}


def _dedup_ldweights(nc):
    """Consecutive matmuls often reuse identical weights (score j-halves, PV
    j-halves, proj q-chunks). The rust layer emits one standalone
    InstLdweights per matmul; reloading identical weights costs ~100ns of PE
    array drain each. Drop an InstLdweights when the previously loaded
    weights AP is byte-identical (weight tiles here are write-once), folding
    its waits into the next matmul (legalized afterwards)."""

    def key(ap):
        return (ap.memref, ap.offset, str(ap.ap), str(ap.dtype))

    n_drop = 0
    for f in nc.m.functions:
        for bb in f.blocks:
            out = []
            pending_waits = []
            last_w = None
            for i in bb.instructions:
                if type(i).__name__ == "InstLdweights":
                    k = key(i.ins[0])
                    if k == last_w:
                        si = getattr(i, "sync_info", None)
                        if si is not None and si.on_wait:
                            pending_waits.extend(si.on_wait)
                        n_drop += 1
                        continue
                    last_w = k
                elif (
                    type(i).__name__ == "InstMatmult"
                    and len(i.ins) > 1
                    and "float32" in str(getattr(i.ins[1], "dtype", ""))
                ):
                    # self-loading fp32 matmul clobbers the PE stationary
                    # registers; forget the cached ldweights AP
                    last_w = None
                    if pending_waits:
                        si = getattr(i, "sync_info", None)
                        waits = list(si.on_wait) if si else []
                        upd = list(si.on_update) if si else []
                        i.sync_info = mybir.SyncInfo(
                            on_wait=pending_waits + waits, on_update=upd)
                        pending_waits = []
                elif pending_waits and type(i).__name__ == "InstMatmult":
                    si = getattr(i, "sync_info", None)
                    waits = list(si.on_wait) if si else []
                    upd = list(si.on_update) if si else []
                    i.sync_info = mybir.SyncInfo(
                        on_wait=pending_waits + waits, on_update=upd)
                    pending_waits = []
                out.append(i)
            assert not pending_waits
            bb.instructions = out
    return n_drop


def _legalize_matmul_waits(nc):
    """walrus allows at most 1 sync wait on engine compute instructions; Tile
    sometimes emits more. Move the excess onto EventSemaphore instructions
    (cap 2 each) placed immediately before in same-engine program order."""
    for f in nc.m.functions:
        for bb in f.blocks:
            out = []
            changed = False
            for i in bb.instructions:
                si = getattr(i, "sync_info", None)
                if (
                    type(i).__name__ in _ENGINE_OPS
                    and si is not None
                    and si.on_wait
                    and len(si.on_wait) > 1
                ):
                    waits = list(si.on_wait)
                    excess, keep = waits[:-1], waits[-1:]
                    for c in range(0, len(excess), 2):
                        ev = mybir.InstEventSemaphore(
                            name=f"{i.name}-mmw{c}", ins=[], outs=[]
                        )
                        ev.engine = i.engine
                        ev.sync_info = mybir.SyncInfo(
                            on_wait=excess[c:c + 2], on_update=[]
                        )
                        out.append(ev)
                    i.sync_info = mybir.SyncInfo(
                        on_wait=keep, on_update=list(si.on_update)
                    )
                    changed = True
                out.append(i)
            if changed:
                bb.instructions = out


def build_nc():
    nc = bass.Bass()

    xt = nc.dram_tensor("xt", [D, S], MMDT, kind="ExternalInput")
    wq = nc.dram_tensor("wq", [D, E], MMDT, kind="ExternalInput")
    wk = nc.dram_tensor("wk", [D, E], MMDT, kind="ExternalInput")
    wv = nc.dram_tensor("wv", [D, E], MMDT, kind="ExternalInput")
    wo = nc.dram_tensor("wo", [E, D], MMDT, kind="ExternalInput")
    bq = nc.dram_tensor("bq", [E], F32, kind="ExternalInput")
    bk = nc.dram_tensor("bk", [E], F32, kind="ExternalInput")
    # bf16 output halves y DMA traffic (the post-attention drain);
    # host accumulates in f32. ~2e-3 rel err, tolerance is 2e-2.
    y = nc.dram_tensor("y", [S, D], BF16, kind="ExternalOutput")

    with tile.TileContext(nc) as tc:
        with tc.tile_pool(name="persist", bufs=1) as pp:
            # ---- persistent tiles ----
            qt_sb = [pp.tile([128, S], MMDT, tag=f"qt{t}", name=f"qt{t}")
                     for t in range(ET)]
            ktp = [pp.tile([128, S], MMDT, tag=f"ktp{t}", name=f"ktp{t}")
                   for t in range(ET)]
            # vstk[:, h, cp, 0:64] = V rows for head h, keys cp*128..+128
            # (partition = key within chunk); [:, h, cp, 64] = 1.0 (rowsum).
            vstk = pp.tile([128, HPC, CP, DK + 1], MMDT, tag="vstk")
            ctx_sb = [pp.tile([128, S], MMDT, tag=f"ctx{t}", name=f"ctx{t}")
                      for t in range(ET)]
            wo_sb = [pp.tile([128, D], MMDT, tag=f"wo{t}", name=f"wo{t}")
                     for t in range(ET)]
            bq_sb = pp.tile([128, ET], F32, tag="bq")
            bk_sb = pp.tile([128, ET], F32, tag="bk")
            xt_sb = [pp.tile([128, S], MMDT, tag=f"xt{k}", name=f"xt{k}")
                     for k in range(KT)]
            wq_sb = pp.tile([128, KT, E], MMDT, tag="wq")
            wk_sb = pp.tile([128, KT, E], MMDT, tag="wk")
            wv_sb = pp.tile([128, KT, E], MMDT, tag="wv")

            # k-layer inputs arrive together so QK0 proj chases the DMAs;
            # xt split over three queues per chunk, wq/wk as single full-E
            # descriptors (512B rows beat 2x256B for packet throughput).
            # (bq/bk issue after the first xt chunk - needed only at ~20us)
            for k in range(KT):
                if k == 0:
                    # pair-0 slice first so the k=0 ldweights fires early
                    nc.scalar.dma_start(wq_sb[:, k, 0:128],
                                        wq[0:128, 0:128])
                    nc.scalar.dma_start(wq_sb[:, k, 128:256],
                                        wq[0:128, 128:256])
                else:
                    nc.scalar.dma_start(wq_sb[:, k, :],
                                        wq[k * 128:(k + 1) * 128, :])
                nc.scalar.dma_start(wk_sb[:, k, :],
                                    wk[k * 128:(k + 1) * 128, :])
                nc.scalar.dma_start(wv_sb[:, k, :],
                                    wv[k * 128:(k + 1) * 128, :])
                if k == 0:
                    nc.sync.dma_start(xt_sb[k][:, 0:512],
                                      xt[k * 128:(k + 1) * 128, 0:512])
                    nc.sync.dma_start(xt_sb[k][:, 512:1024],
                                      xt[k * 128:(k + 1) * 128, 512:1024])
                else:
                    nc.sync.dma_start(xt_sb[k][:, 0:1024],
                                      xt[k * 128:(k + 1) * 128, 0:1024])
                nc.gpsimd.dma_start(xt_sb[k][:, 1024:2048],
                                    xt[k * 128:(k + 1) * 128, 1024:2048])
            # bq/bk are tiny elementwise scatters; keep them off the xt
            # path (needed only at the first QK eviction, ~30us)
            nc.gpsimd.dma_start(bq_sb, bq.rearrange("(t p) -> p t", p=128))
            nc.gpsimd.dma_start(bk_sb, bk.rearrange("(t p) -> p t", p=128))
            # warm the ACT exp table while DMAs stream: the compiler
            # statically inserts the 1.3us EXP table load before the first
            # exp-consuming ACT instruction; this dummy (queued before the
            # wv/wo DMA issues) pulls that load off the phase-A critical
            # path.
            ones_sb2 = pp.tile([128, CP], F32, tag="ones2")
            nc.vector.memset(ones_sb2, 1.0)
            tbl_warm = pp.tile([128, 16], MMDT, tag="tblw")
            nc.scalar.activation(tbl_warm, ones_sb2[:, 0:16], AF.Exp,
                                 scale=0.125)
            for t in range(ET):
                nc.scalar.dma_start(wo_sb[t], wo[t * 128:(t + 1) * 128, :])

            ones_sb = pp.tile([128, CP], F32, tag="ones")
            nc.vector.memset(ones_sb, 1.0)
            for h in range(HPC):
                nc.vector.tensor_copy(vstk[:, h, :, DK:DK + 1],
                                      ones_sb[:, :, None])
            # one-time tiles for the final-section PE-based normalize chain
            # (see section_end): identities for the two transposes (id32
            # also copied to base partition 64 - matmul requires lhsT/rhs
            # at the same base), per-chunk one-hot weights for the
            # q-broadcast matmul, and staging tiles.
            id32 = pp.tile([32, 32], MMDT, tag="id32")
            nc.vector.memset(id32, 1.0)
            nc.gpsimd.affine_select(id32, id32, pattern=[[-1, 32]],
                                    compare_op=mybir.AluOpType.is_equal,
                                    fill=0.0, base=0, channel_multiplier=1)
            id32_hi = pp.tile([96, 32], MMDT, tag="id32hi")
            nc.sync.dma_start(id32_hi[64:96, :], id32)
            id128 = pp.tile([128, 128], MMDT, tag="id128")
            nc.vector.memset(id128, 1.0)
            nc.gpsimd.affine_select(id128, id128, pattern=[[-1, 128]],
                                    compare_op=mybir.AluOpType.is_equal,
                                    fill=0.0, base=0, channel_multiplier=1)
            onesW = pp.tile([DK, 8 * DK], MMDT, tag="onesW")
            nc.vector.memset(onesW, 1.0)
            nc.gpsimd.affine_select(
                onesW.rearrange("p (c f) -> p c f", f=DK),
                onesW.rearrange("p (c f) -> p c f", f=DK),
                pattern=[[-1, 8], [0, DK]],
                compare_op=mybir.AluOpType.is_equal,
                fill=0.0, base=0, channel_multiplier=1)
            rinvT_sb = pp.tile([128, 128], MMDT, tag="rinvT")
            nc.vector.memset(rinvT_sb, 0.0)
            # final-section staging: ctx rows 0:64 + rowsum row 64; rows
            # 65:96 zeroed so the 32-row transpose slab is NaN-free.
            stgF = pp.tile([96, 1024], MMDT, tag="stg16")
            # rows 64:96 zeroed at setup; row 64 is overwritten with the
            # rowsum by the final section's staging copies.
            nc.vector.memset(stgF[DK:96, :], 0.0)

            # ---- stage A: pair-0 Q/K projections + V (all heads) ----
            with tc.tile_pool(name="psA", bufs=4, space="PSUM") as psA:
                # 4 resident [128,1024] psums (8 banks), k-outer so matmuls
                # chase the input DMAs.
                qps = [psA.tile([128, 1024], F32, tag="proj", name=f"qp{i}")
                       for i in range(2)]
                kps = [psA.tile([128, 1024], F32, tag="proj", name=f"kp{i}")
                       for i in range(2)]
                for k in range(KT):
                    for ps, w_sb in ((qps, wq_sb), (kps, wk_sb)):
                        for half in range(2):
                            for j in range(2):
                                q0 = half * 1024 + j * 512
                                nc.tensor.matmul(
                                    ps[half][:, j * 512:(j + 1) * 512],
                                    w_sb[:, k, 0:128],
                                    xt_sb[k][:, q0:q0 + 512],
                                    start=(k == 0), stop=(k == KT - 1),
                                )
                # evictions with bias on DVE (idle during stage A; ACT
                # carries the prefetched exp-table load)
                for half in range(2):
                    nc.vector.tensor_scalar_add(
                        qt_sb[0][:, half * 1024:(half + 1) * 1024],
                        qps[half], bq_sb[:, 0:1])
                for half in range(2):
                    nc.vector.tensor_scalar_add(
                        ktp[0][:, half * 1024:(half + 1) * 1024],
                        kps[half], bk_sb[:, 0:1])

                # V for all 4 heads, seq-major: psum [128 seq, 256 feat].
                # (A feature-major V + DMA-xbar transpose into vstk was
                # tried - fewer ldweights - but the xbar's output run
                # ordering for free dims > 256 doesn't match its documented
                # tile layout, and stage A is HBM-bound anyway.)
                for s in range(CP):
                    vp = psA.tile([128, 1024], F32, tag="proj",
                                  name=f"vp{s}")
                    for k in range(KT):
                        nc.tensor.matmul(
                            vp[:, 0:E],
                            xt_sb[k][:, s * 128:(s + 1) * 128],
                            wv_sb[:, k, :],
                            start=(k == 0), stop=(k == KT - 1),
                        )
                    nc.vector.tensor_copy(
                        vstk[:, :, s, 0:DK],
                        vp[:, 0:E].rearrange("p (h d) -> p h d", d=DK))

            # ---- stage B: one pipelined attention loop ----
            pb = pp  # stage-B sbuf tiles live in the persist pool
            with (
                tc.tile_pool(name="dramB", bufs=3, space="DRAM") as dramB,
            ):
              with tc.tile_pool(name="psB", bufs=2,
                                space="PSUM") as psB:
                psS = psC = psX = psB
                # --- feeder jobs: closures emitting PE matmuls ---
                def proj1_jobs(w_sb, b_col, dst, pairs):
                    """Pair-1 proj matmuls, k-inner per [128,1024] chunk
                    pair; the two 512-col matmuls per k share one
                    ldweights."""
                    jobs = []
                    for cp0 in pairs:
                        ps = {}

                        def mk(k, cp0=cp0, ps=ps):
                            def go():
                                if k == 0:
                                    ps[0] = psX.tile(
                                        [128, 1024], F32, tag="aux",
                                        name=f"pj{id(w_sb)}_{cp0}", bufs=1)
                                for j in range(2):
                                    nc.tensor.matmul(
                                        ps[0][:, j * 512:(j + 1) * 512],
                                        w_sb[:, k, 128:256],
                                        xt_sb[k][:, cp0 * 1024 + j * 512:
                                                  cp0 * 1024 + (j + 1) * 512],
                                        start=(k == 0), stop=(k == KT - 1),
                                    )
                                if k == KT - 1:
                                    nc.vector.tensor_scalar_add(
                                        dst[:, cp0 * 1024:(cp0 + 1) * 1024],
                                        ps[0], b_col)
                            return go
                        for k in range(KT):
                            jobs.append(mk(k))
                    return jobs

                def outproj_jobs(tiles):
                    """Output projection per 128-q tile: 2 jobs of 2 matmuls
                    (both n-halves share the ctx-chunk ldweights) + evict +
                    y DMA."""
                    jobs = []
                    for qt_i in tiles:
                        st = {}

                        def mk(t, qt_i=qt_i, st=st):
                            def go():
                                if t == 0:
                                    st['ps'] = psX.tile(
                                        [128, 1024], F32, tag="aux",
                                        name=f"yp{qt_i}", bufs=1)
                                    st['ys'] = pb.tile(
                                        [128, 1024], BF16, tag="ys",
                                        name=f"ys{qt_i}", bufs=6)
                                for n in range(2):
                                    nc.tensor.matmul(
                                        st['ps'][:, n * 512:(n + 1) * 512],
                                        ctx_sb[t][:,
                                                  qt_i * 128:(qt_i + 1) * 128],
                                        wo_sb[t][:, n * 512:(n + 1) * 512],
                                        start=(t == 0), stop=(t == ET - 1),
                                    )
                                if t == ET - 1:
                                    nc.vector.tensor_copy(st['ys'], st['ps'])
                                    nc.sync.dma_start(
                                        y[qt_i * 128:(qt_i + 1) * 128, :],
                                        st['ys'])
                            return go
                        for t in range(ET):
                            jobs.append(mk(t))
                    return jobs

                # schedule feeder jobs onto global iterations
                schedule = [[] for _ in range(NG)]

                def assign(g_lo, g_hi, jobs):
                    n_slots = g_hi - g_lo
                    acc = 0.0
                    i = 0
                    per = len(jobs) / n_slots
                    for g in range(g_lo, g_hi):
                        acc += per
                        take = int(round(acc)) - i
                        schedule[g] = jobs[i:i + take]
                        i += take
                    assert i == len(jobs)

                # Q1 half0 + K1 during sections (0,0),(1,0); order matters:
                # section (2,0)'s first scores (emitted at g=31) need the
                # Q1-h0 and K1-ch0 evictions done well before.
                assign(0, 32,
                       proj1_jobs(wq_sb, bq_sb[:, 1:2], qt_sb[1], [0])
                       + proj1_jobs(wk_sb, bk_sb[:, 1:2], ktp[1], [0, 1]))
                # Q1 half1 during (2,0),(3,0)
                assign(32, 64,
                       proj1_jobs(wq_sb, bq_sb[:, 1:2], qt_sb[1], [1]))
                # outproj q-half0, all 8 tiles, during (0,1)..(2,1); offset
                # past (3,0)'s normalize chain (its ctx lands a few us into
                # (0,1)).
                assign(72, 126, outproj_jobs(range(8)))

                def emit_scores(g):
                    sec, cp = divmod(g, CP)
                    h, qh = SECTIONS[sec]
                    t, hp = h // 2, h % 2
                    sc_ps = psS.tile([128, 1024], F32, tag="sc",
                                     name=f"sc{g}")
                    for j in range(2):
                        nc.tensor.matmul(
                            sc_ps[:, j * 512:(j + 1) * 512],
                            ktp[t][hp * 64:hp * 64 + 64,
                                   cp * 128:(cp + 1) * 128],
                            qt_sb[t][hp * 64:hp * 64 + 64,
                                     qh * 1024 + j * 512:
                                     qh * 1024 + (j + 1) * 512],
                            start=True, stop=True,
                        )
                    return sc_ps

                def section_end(h, qh, ctx_ps, final=False):
                    t, hp = h // 2, h % 2
                    if final:
                        # Last section: the DMA-hop chain below would sit
                        # exposed on the critical path (~13us of elementwise
                        # DMA latency in a congested tail). PE/DVE-only
                        # instead: transpose the 32-row rowsum slab (row 64
                        # = rowsum, 65:96 zeros) onto q-partitions,
                        # reciprocal on 128 lanes, transpose back to a row
                        # layout, then contraction-64 matmuls with one-hot
                        # weights broadcast rinv[q] to the 64 ctx
                        # partitions.
                        for j in range(2):
                            nc.vector.tensor_copy(
                                stgF[0:DK + 1, j * 512:(j + 1) * 512],
                                ctx_ps[j])
                        rsT = psX.tile([128, 1024], MMDT, tag="aux",
                                       name="rsT", bufs=1)
                        for c in range(8):
                            nc.tensor.transpose(
                                rsT[:, c * 32:(c + 1) * 32],
                                stgF[DK:DK + 32, c * 128:(c + 1) * 128],
                                id32_hi[DK:DK + 32, :])
                        rinvq = pb.tile([128, 8], MMDT, tag="rinvq")
                        with nc.allow_low_precision("fp16 rinv, tol 2e-2"):
                            nc.vector.reciprocal(
                                rinvq[:, :, None],
                                rsT[:, 0:256].rearrange(
                                    "p (c w) -> p c w", w=32)[:, :, 0:1])
                        rinvT_ps = psX.tile([128, 1024], MMDT, tag="aux",
                                            name="rinvT_ps", bufs=1)
                        nc.tensor.transpose(rinvT_ps[0:8, 0:128], rinvq,
                                            id128)
                        nc.vector.tensor_copy(rinvT_sb[0:8, :],
                                              rinvT_ps[0:8, 0:128])
                        rb_ps = psX.tile([128, 1024], F32, tag="aux",
                                         name="rbps", bufs=1)
                        for c in range(8):
                            nc.tensor.matmul(
                                rb_ps[0:DK, c * 128:(c + 1) * 128],
                                onesW[:, c * DK:(c + 1) * DK],
                                rinvT_sb[0:DK, 0:128],
                                start=True, stop=True)
                        for j in range(2):
                            nc.vector.tensor_mul(
                                ctx_sb[t][hp * 64:hp * 64 + 64,
                                          qh * 1024 + j * 512:
                                          qh * 1024 + (j + 1) * 512],
                                stgF[0:DK, j * 512:(j + 1) * 512],
                                rb_ps[0:DK, j * 512:(j + 1) * 512],
                            )
                        return
                    stg = pb.tile([DK + 1, 1024], F32, tag="stg",
                                  name=f"stg{h}_{qh}", bufs=3)
                    for j in range(2):
                        nc.vector.tensor_copy(
                            stg[:, j * 512:(j + 1) * 512], ctx_ps[j])
                    # reciprocal of rowsum via DRAM scatter to 64 partitions
                    # ([1,1024] single-lane DVE reciprocal is ~6.5us; this
                    # chain is ~3us and hidden under the next section).
                    rc_dr = dramB.tile([1, 1024], F32, tag="rc_dr",
                                       name=f"rcdr{h}_{qh}")
                    # single SBUF->SBUF partition-scatter DMA (dst/src APs
                    # iterate element-wise) replaces the two-hop DRAM bounce
                    rs64 = pb.tile([64, 16], F32, tag="rs64",
                                   name=f"rs64{h}_{qh}", bufs=3)
                    nc.sync.dma_start(rs64, stg[DK:DK + 1, :])
                    rc64 = pb.tile([64, 16], F32, tag="rc64",
                                   name=f"rc64{h}_{qh}", bufs=3)
                    nc.vector.reciprocal(rc64, rs64)
                    nc.sync.dma_start(
                        rc_dr.rearrange("o (p f) -> (o p) f", f=16), rc64)
                    rb = pb.tile([64, 1024], F32, tag="rb",
                                 name=f"rb{h}_{qh}", bufs=3)
                    # halves pipelined: first mul starts after half the
                    # broadcast transfer (matters for the final section,
                    # whose chain gates the tail)
                    for j in range(2):
                        nc.sync.dma_start(
                            rb[:, j * 512:(j + 1) * 512],
                            rc_dr[:, j * 512:(j + 1) * 512]
                            .to_broadcast([64, 512]))
                        nc.vector.tensor_mul(
                            ctx_sb[t][hp * 64:hp * 64 + 64,
                                      qh * 1024 + j * 512:
                                      qh * 1024 + (j + 1) * 512],
                            stg[0:DK, j * 512:(j + 1) * 512],
                            rb[:, j * 512:(j + 1) * 512],
                        )

                ctx_ps = None
                sc_cur = emit_scores(0)
                for g in range(NG):
                    sec, cp = divmod(g, CP)
                    h, qh = SECTIONS[sec]
                    if cp == 0:
                        ctx_ps = [psC.tile([DK + 1, 512], F32, tag="ctx",
                                           name=f"cx{sec}_{j}")
                                  for j in range(2)]
                    sc_next = emit_scores(g + 1) if g + 1 < NG else None
                    ex = pb.tile([128, 1024], MMDT, tag="ex",
                                 name=f"ex{g}", bufs=6)
                    nc.scalar.activation(ex, sc_cur, AF.Exp, scale=0.125)
                    for job in schedule[g]:
                        job()
                    for j in range(2):
                        nc.tensor.matmul(
                            ctx_ps[j],
                            vstk[:, h, cp, :],
                            ex[:, j * 512:(j + 1) * 512],
                            start=(cp == 0), stop=(cp == CP - 1),
                        )
                    if cp == CP - 1 and g < NG - 1:
                        section_end(h, qh, ctx_ps)
                    sc_cur = sc_next

                # ---- tail: output projection for q-half 1, inside the
                # psB scope so y psums ride the freed sc-tag rotation.
                # Tiles 8-9's t=0 matmuls are emitted before the final
                # normalize chain so the PE stays busy (and in its high
                # p-state) while the chain's reciprocal resolves; tiles
                # 10-15 then run full psum chains.
                def t_mm(yp, t, qt_i, start, stop):
                    for n in range(2):
                        nc.tensor.matmul(
                            yp[:, n * 512:(n + 1) * 512],
                            ctx_sb[t][:, qt_i * 128:(qt_i + 1) * 128],
                            wo_sb[t][:, n * 512:(n + 1) * 512],
                            start=start, stop=stop,
                        )

                yps = {}
                for qt_i in range(8, 10):
                    yp = psS.tile([128, 1024], F32, tag="sc",
                                  name=f"yt{qt_i}")
                    yps[qt_i] = yp
                    t_mm(yp, 0, qt_i, start=True, stop=False)

                section_end(3, 1, ctx_ps, final=True)

                def evict_dma(qt_i, yp, i):
                    ys = pb.tile([128, 1024], BF16, tag="ys",
                                 name=f"ys{qt_i}", bufs=6)
                    if i % 2 == 0:
                        nc.scalar.copy(ys, yp)
                    else:
                        nc.vector.tensor_copy(ys, yp)
                    rows = slice(qt_i * 128, (qt_i + 1) * 128)
                    qs = (nc.sync, nc.gpsimd, nc.scalar)
                    qs[qt_i % 3].dma_start(y[rows, 0:512], ys[:, 0:512])
                    qs[(qt_i + 1) % 3].dma_start(y[rows, 512:1024],
                                                 ys[:, 512:1024])

                for i, qt_i in enumerate(range(8, 10)):
                    yp = yps[qt_i]
                    t_mm(yp, 1, qt_i, start=False, stop=True)
                    evict_dma(qt_i, yp, i)
                for i, qt_i in enumerate(range(10, 15)):
                    yp = psS.tile([128, 1024], F32, tag="sc",
                                  name=f"yt{qt_i}")
                    t_mm(yp, 0, qt_i, start=True, stop=False)
                    t_mm(yp, 1, qt_i, start=False, stop=True)
                    evict_dma(qt_i, yp, i)
                # last tile: evict+DMA per 512-col half on separate engines
                # so the final transfer starts as early as possible
                yp = psS.tile([128, 1024], F32, tag="sc", name="yt15")
                t_mm(yp, 0, 15, start=True, stop=False)
                t_mm(yp, 1, 15, start=False, stop=True)
                ys15 = pb.tile([128, 1024], BF16, tag="ys", name="ys15",
                               bufs=6)
                nc.scalar.copy(ys15[:, 0:512], yp[:, 0:512])
                nc.sync.dma_start(y[15 * 128:2048, 0:512], ys15[:, 0:512])
                nc.vector.tensor_copy(ys15[:, 512:1024], yp[:, 512:1024])
                nc.gpsimd.dma_start(y[15 * 128:2048, 512:1024],
                                    ys15[:, 512:1024])
    _dedup_ldweights(nc)
    _legalize_matmul_waits(nc)
    return nc


_NC_CACHE = None


def _get_nc():
    global _NC_CACHE
    if _NC_CACHE is None:
        _NC_CACHE = build_nc()
    return _NC_CACHE


def make_in_maps(inputs):
    mmnp = mybir.dt.np(MMDT)
    x = np.asarray(inputs["x"], dtype=np.float32)
    Wq = np.asarray(inputs["Wq"], dtype=np.float32)
    Wk = np.asarray(inputs["Wk"], dtype=np.float32)
    Wv = np.asarray(inputs["Wv"], dtype=np.float32)
    Wo = np.asarray(inputs["Wo"], dtype=np.float32)
    bq = np.asarray(inputs["bq"], dtype=np.float32)
    bk = np.asarray(inputs["bk"], dtype=np.float32)

    in_maps = []
    for c in range(N_CORES):
        b, g = c // 4, c % 4
        sl = slice(g * E, (g + 1) * E)
        in_maps.append({
            "xt": np.ascontiguousarray(x[b].T).astype(mmnp),
            "wq": np.ascontiguousarray(Wq[:, sl]).astype(mmnp),
            "wk": np.ascontiguousarray(Wk[:, sl]).astype(mmnp),
            "wv": np.ascontiguousarray(Wv[:, sl]).astype(mmnp),
            "wo": np.ascontiguousarray(Wo[sl, :]).astype(mmnp),
            "bq": np.ascontiguousarray(bq[sl]),
            "bk": np.ascontiguousarray(bk[sl]),
        })
    return in_maps


def kernel(x, Wq, bq, Wk, bk, Wv, bv, Wo, bo):
    from concourse.bass_utils import run_bass_kernel_spmd

    x = np.asarray(x, dtype=np.float32)
    Wv = np.asarray(Wv, dtype=np.float32)
    Wo = np.asarray(Wo, dtype=np.float32)
    bv = np.asarray(bv, dtype=np.float32)
    bo = np.asarray(bo, dtype=np.float32)

    B = x.shape[0]
    nc = _get_nc()
    in_maps = make_in_maps({
        "x": x, "Wq": Wq, "Wk": Wk, "Wv": Wv, "Wo": Wo, "bq": bq, "bk": bk,
    })

    res = run_bass_kernel_spmd(nc, in_maps, core_ids=list(range(N_CORES)))

    bias_total = bo + bv @ Wo  # [D]
    out = np.zeros((B, S, D), dtype=np.float32)
    for c in range(N_CORES):
        out[c // 4] += np.asarray(res.results[c]["y"], dtype=np.float32)
    out += bias_total[None, None, :]
    return out

